# revision 1
# baseline (speedup 1.0000x reference)
"""FBSNN loss kernel for Trainium2 (8 NeuronCores, pure data parallel).

Reference computation: 50-step Euler scheme over B=262144 paths; each step
evaluates a Q-control MLP and a Y-value MLP (2->64->64->1, relu) plus the
dY/dy JVP, accumulating mean squared BSDE residuals and a terminal loss.

Kernel architecture (per core, Bc = 32768 paths = 64 chunks x 512):
  - All per-path state (y, dt*q, u, du) kept DENSE as [64, 512] SBUF tiles
    (partition = chunk, free = path-within-chunk).
  - L1 consumes the dense y tile directly: lhsT is a one-hot-row [64, 128]
    stationary operand (row c = fused [Yw1col|Qw1col]) so a K=64 matmul
    extracts chunk c's y row while computing the outer product.
  - L3 produces dense outputs directly: one-hot-column stationary operands
    write chunk c's result into psum partition row c, accumulating across
    chunks into one [64/128, 512] psum tile (start only on chunk 0).
  - Y/Q nets fused into one 128-wide evaluation (both evaluated at the same
    (t, y) point by restructuring the recurrence); relu-jvp folded into
    precomputed weights; all matmuls run as float32r (1 cycle/row).
  - Dynamic For_i loop over 25 iterations x 2 steps (ping-pong parity).
  - Loss accumulated on-chip via fused tensor_tensor_reduce into [64, 1];
    per-core partial sums combined on host (no collectives).
"""

import os
import sys

import numpy as np

sys.path.insert(0, "/opt/trn_rl_repo")

import concourse.bacc as bacc  # noqa: E402
import concourse.bass as bass  # noqa: E402
import concourse.tile as tile  # noqa: E402
from concourse import mybir  # noqa: E402
from concourse.bass_utils import run_bass_kernel_spmd  # noqa: E402

DT = 0.01
SIGMA = 0.5
N_STEPS = 50
N_CORES = 8
B_TOTAL = 262144
N_CHUNKS = 64
NFREE = 512
BC = N_CHUNKS * NFREE  # paths per core

F32 = mybir.dt.float32
F32R = mybir.dt.float32r
RELU = mybir.ActivationFunctionType.Relu
SQUARE = mybir.ActivationFunctionType.Square
COPY = mybir.ActivationFunctionType.Copy
ADD = mybir.AluOpType.add
SUB = mybir.AluOpType.subtract
MULT = mybir.AluOpType.mult
IS_GT = mybir.AluOpType.is_gt

_CACHE = {}
_LAST_RES = None
UNROLL = bool(int(os.environ.get("FBSNN_UNROLL", "1")))
ABL = os.environ.get("FBSNN_ABL", "")


def _r(ap):
    """float32r view of an fp32 AP (full-rate PE matmul)."""
    return ap.bitcast(F32R)


def _build(n_steps, n_chunks, nfree, y0val):
    """Trace the Bass kernel. Returns the Bass object."""
    nc = bacc.Bacc(None, target_bir_lowering=False)

    H = 128  # fused hidden width (64 Y + 64 Q)
    HH = 64

    # ---- DRAM I/O ----
    dws = nc.dram_tensor("dws", [n_steps, n_chunks, nfree], F32, kind="ExternalInput")
    l1c = nc.dram_tensor("l1c", [n_chunks, n_chunks * H], F32R, kind="ExternalInput")
    w2d = nc.dram_tensor("w2d", [H, H], F32R, kind="ExternalInput")
    w2p = nc.dram_tensor("w2p", [HH, HH], F32R, kind="ExternalInput")
    uc3 = nc.dram_tensor("uc3", [HH, n_chunks * n_chunks], F32R, kind="ExternalInput")
    qc3 = nc.dram_tensor("qc3", [H, n_chunks * n_chunks], F32R, kind="ExternalInput")
    c1t = nc.dram_tensor("c1t", [H, 64], F32, kind="ExternalInput")
    b2t = nc.dram_tensor("b2t", [H, 1], F32, kind="ExternalInput")
    scal = nc.dram_tensor("scal", [128, 8], F32, kind="ExternalInput")
    loss_out = nc.dram_tensor("loss_part", [n_chunks, 1], F32, kind="ExternalOutput")

    with tile.TileContext(nc) as tc:
        import contextlib

        with contextlib.ExitStack() as ctx:
            const = ctx.enter_context(tc.tile_pool(name="const", bufs=1))
            state = ctx.enter_context(tc.tile_pool(name="state", bufs=1))
            work = ctx.enter_context(tc.tile_pool(name="work", bufs=3))
            dwp = ctx.enter_context(tc.tile_pool(name="dwp", bufs=3))
            scr = ctx.enter_context(tc.tile_pool(name="scr", bufs=2))
            ps_a1 = ctx.enter_context(tc.tile_pool(name="ps_a1", bufs=2, space="PSUM"))
            ps_a2 = ctx.enter_context(tc.tile_pool(name="ps_a2", bufs=2, space="PSUM"))
            ps_v = ctx.enter_context(tc.tile_pool(name="ps_v", bufs=1, space="PSUM"))
            ps_qd = ctx.enter_context(tc.tile_pool(name="ps_qd", bufs=1, space="PSUM"))
            ps_u = ctx.enter_context(tc.tile_pool(name="ps_u", bufs=1, space="PSUM"))
            ps_du = ctx.enter_context(tc.tile_pool(name="ps_du", bufs=1, space="PSUM"))

            # ---- load constants to SBUF ----
            l1c_sb = const.tile([n_chunks, n_chunks * H], F32R)
            w2d_sb = const.tile([H, H], F32R)
            w2p_sb = const.tile([HH, HH], F32R)
            uc3_sb = const.tile([HH, n_chunks * n_chunks], F32R)
            qc3_sb = const.tile([H, n_chunks * n_chunks], F32R)
            c1t_sb = const.tile([H, 64], F32)
            b2t_sb = const.tile([H, 1], F32)
            scal_sb = const.tile([128, 8], F32)
            nc.sync.dma_start(l1c_sb[:], l1c[:])
            nc.sync.dma_start(w2d_sb[:], w2d[:])
            nc.sync.dma_start(w2p_sb[:], w2p[:])
            nc.sync.dma_start(uc3_sb[:], uc3[:])
            nc.sync.dma_start(qc3_sb[:], qc3[:])
            nc.sync.dma_start(c1t_sb[:], c1t[:])
            nc.sync.dma_start(b2t_sb[:], b2t[:])
            nc.sync.dma_start(scal_sb[:], scal[:])

            # ---- persistent state ----
            y_sb = state.tile([n_chunks, nfree], F32)  # y - n*qb (shifted state)
            qd_sb = [state.tile([n_chunks, nfree], F32, tag=f"qd{p}", name=f"qd{p}") for p in (0, 1)]
            u_st = [state.tile([n_chunks, nfree], F32, tag=f"u{p}", name=f"u{p}") for p in (0, 1)]
            du_st = [state.tile([n_chunks, nfree], F32, tag=f"du{p}", name=f"du{p}") for p in (0, 1)]
            acc = state.tile([n_chunks, 1], F32, tag="acc", name="acc")
            sacc = state.tile([n_chunks, 1], F32, tag="sacc", name="sacc")

            y_init = scr.tile([n_chunks, nfree], F32, tag="sq", name="y_init")
            nc.vector.memset(y_init[:], float(y0val))
            nc.scalar.activation(_r(y_sb[:]), y_init[:], COPY)
            nc.vector.memset(acc[:], 0.0)

            def emit_eval(bias_col, qd_dst, u_dst, du_dst):
                """Fused Y+Q MLP eval (+ Y jvp) on dense y_sb.

                bias_col: AP [H, 1] slice of c1t_sb for this step's L1 bias.
                Writes dt*q / u / du as dense [C, nfree] sbuf tiles.
                """
                C = n_chunks
                qd_ps = ps_qd.tile([C, nfree], F32, tag="qd_ps", name="qd_ps")
                u_ps = ps_u.tile([C, nfree], F32, tag="u_ps", name="u_ps")
                du_ps = ps_du.tile([C, nfree], F32, tag="du_ps", name="du_ps")
                for c in range(C):
                    a1 = ps_a1.tile([H, nfree], F32, tag="a1", name="a1")
                    a2 = ps_a2.tile([H, nfree], F32, tag="a2", name="a2")
                    v = ps_v.tile([HH, nfree], F32, tag="v", name="v")
                    h1 = work.tile([H, nfree], F32, tag="h1", name="h1")
                    h2 = work.tile([H, nfree], F32, tag="h2", name="h2")
                    m1 = work.tile([HH, nfree], F32, tag="m1", name="m1")
                    m2 = work.tile([HH, nfree], F32, tag="m2", name="m2")
                    dh2 = work.tile([HH, nfree], F32, tag="dh2", name="dh2")

                    # L1: a1 = (onehot_row(c) x w1col)^T @ y   (K=C)
                    nc.tensor.matmul(
                        a1[:], l1c_sb[:, c * H : (c + 1) * H], _r(y_sb[:])
                    )
                    # h1 = relu(a1 + c1[:, n])
                    nc.scalar.activation(_r(h1[:]), a1[:], RELU, bias=bias_col)
                    # mask1 = (h1_Y > 0)
                    mask_eng = nc.vector if "dvemask" in ABL else nc.gpsimd
                    mask_eng.tensor_scalar(_r(m1[:]), h1[:HH, :], 0.0, None, IS_GT)
                    # L2: a2 = blockdiag(Yw2, Qw2)^T @ h1
                    nc.tensor.matmul(a2[:], w2d_sb[:], _r(h1[:]))
                    for _d in range(ABL.count("duppe")):
                        nc.tensor.matmul(a2[:], w2d_sb[:], _r(h1[:]))
                    # L2 jvp: v = (diag(Yw1col) Yw2)^T @ mask1
                    if "nodu" not in ABL:
                        nc.tensor.matmul(v[:], w2p_sb[:], _r(m1[:]))
                    # h2 = relu(a2 + b2)
                    nc.scalar.activation(_r(h2[:]), a2[:], RELU, bias=b2t_sb[:, 0:1])
                    # L3 q: accumulate dt*q into row c of qd_ps
                    nc.tensor.matmul(
                        qd_ps[:],
                        qc3_sb[HH:H, c * C : (c + 1) * C],
                        _r(h2[HH:H, :]),
                        start=(c == 0),
                        stop=(c == C - 1),
                        skip_group_check=True,
                    )
                    # mask2 = (h2_Y > 0); dh2 = v * mask2
                    if "nodu" not in ABL:
                        mask_eng.tensor_scalar(m2[:], h2[:HH, :], 0.0, None, IS_GT)
                        nc.vector.tensor_tensor(_r(dh2[:]), v[:], m2[:], MULT)
                        for _d in range(ABL.count("dupdve")):
                            nc.vector.tensor_tensor(_r(dh2[:]), v[:], m2[:], MULT)
                    # L3 u: accumulate u into row c of u_ps
                    nc.tensor.matmul(
                        u_ps[:],
                        uc3_sb[:, c * C : (c + 1) * C],
                        _r(h2[:HH, :]),
                        start=(c == 0),
                        stop=(c == C - 1),
                        skip_group_check=True,
                    )
                    # L3 du: accumulate du into row c of du_ps
                    if "nodu" not in ABL:
                        nc.tensor.matmul(
                            du_ps[:],
                            uc3_sb[:, c * C : (c + 1) * C],
                            _r(dh2[:]),
                            start=(c == 0),
                            stop=(c == C - 1),
                            skip_group_check=True,
                        )
                # evict psum -> sbuf
                nc.scalar.activation(qd_dst[:], qd_ps[:], COPY)
                nc.scalar.activation(u_dst[:], u_ps[:], COPY)
                if "nodu" not in ABL:
                    nc.scalar.activation(du_dst[:], du_ps[:], COPY)
                else:
                    nc.scalar.activation(du_dst[:], u_ps[:], COPY)

            # scal layout: [sq_scale, sq_bias, qb*n_steps, 0...]
            sq_scale = scal_sb[:n_chunks, 0:1]

            def emit_step(n_expr, bias_col, par):
                """One recurrence step: y update, eval at (t_{n+1}, y_{n+1}),
                residual accumulate. par = parity of n (src buffers)."""
                src, dst = par, 1 - par
                if bias_col is None:
                    # dynamic step index: stage c1t column via DMA (register
                    # offsets on ACT bias operands don't work on silicon)
                    bias_t = dwp.tile([H, 1], F32, tag="bias", name="bias_t")
                    nc.gpsimd.dma_start(
                        bias_t[:], c1t_sb[:, bass.ds(n_expr + 1, 1)])
                    bias_col = bias_t[:, 0:1]
                dw_t = dwp.tile([n_chunks, nfree], F32, tag="dw")
                dma_eng = nc.sync if UNROLL else nc.gpsimd
                if isinstance(n_expr, int):
                    dma_eng.dma_start(dw_t[:], dws[n_expr, :, :])
                else:
                    dma_eng.dma_start(dw_t[:], dws[bass.ds(n_expr, 1), :, :])
                # y += dt*q ; y += sigma*dW   (qb drift folded into c1t)
                nc.vector.tensor_tensor(_r(y_sb[:]), y_sb[:], qd_sb[src][:], ADD)
                nc.vector.tensor_tensor(_r(y_sb[:]), y_sb[:], dw_t[:], ADD)
                # eval at new point
                emit_eval(bias_col, qd_sb[dst], u_st[dst], du_st[dst])
                # resid = (u1 - u0) + (0.5/dt)*(qd+qb)^2 - du0*dWs
                sq = scr.tile([n_chunks, nfree], F32, tag="sq")
                r1 = scr.tile([n_chunks, nfree], F32, tag="r1")
                r3 = scr.tile([n_chunks, nfree], F32, tag="r3")
                rr = scr.tile([n_chunks, nfree], F32, tag="rr")
                # sq = (qd*s + qb*s)^2 via ACT Square(scale, bias)
                nc.scalar.activation(
                    sq[:], qd_sb[src][:], SQUARE,
                    bias=scal_sb[:n_chunks, 1:2], scale=sq_scale,
                )
                nc.vector.tensor_tensor(r1[:], u_st[dst][:], u_st[src][:], SUB)
                nc.vector.tensor_tensor(r3[:], du_st[src][:], dw_t[:], MULT)
                nc.vector.tensor_tensor(r1[:], r1[:], sq[:], ADD)
                nc.vector.tensor_tensor(r1[:], r1[:], r3[:], SUB)
                # acc += sum_f(resid^2): ACT Square w/ accum_out + tiny add
                nc.scalar.activation(rr[:], r1[:], SQUARE, accum_out=sacc[:])
                nc.vector.tensor_tensor(acc[:], acc[:], sacc[:], ADD)

            # ---- E_0 at (t_0, y_0) ----
            emit_eval(c1t_sb[:, 0:1], qd_sb[0], u_st[0], du_st[0])

            # ---- main loop ----
            if UNROLL:
                for n in range(n_steps):
                    emit_step(n, c1t_sb[:, n + 1 : n + 2], n % 2)
            else:
                n_iters = n_steps // 2
                if n_iters > 0:
                    with tc.For_i(0, 2 * n_iters, 2) as i:
                        emit_step(i, None, 0)
                        emit_step(i + 1, None, 1)
                for n in range(2 * (n_steps // 2), n_steps):  # odd leftover
                    emit_step(n, c1t_sb[:, n + 1 : n + 2], n % 2)

            fin = n_steps % 2  # parity of final buffers
            # ---- terminal: acc += (u_N - y_N^2)^2 ----
            # y_N = y_sb + n_steps*qb ; term = u_N - y_N^2
            t1 = scr.tile([n_chunks, nfree], F32, tag="sq")
            t2 = scr.tile([n_chunks, nfree], F32, tag="r1")
            rr = scr.tile([n_chunks, nfree], F32, tag="rr")
            # t1 = (y + n*qb)^2
            nc.scalar.activation(t1[:], y_sb[:], SQUARE, bias=scal_sb[:n_chunks, 2:3])
            nc.vector.tensor_tensor(t2[:], u_st[fin][:], t1[:], SUB)
            # + Yb3 cancels only in differences; terminal uses absolute u:
            # u_true = u_psum + Yb3 -> add via tensor_scalar
            nc.vector.tensor_scalar(t2[:], t2[:], scal_sb[:n_chunks, 3:4], None, ADD)
            nc.scalar.activation(rr[:], t2[:], SQUARE, accum_out=sacc[:])
            nc.vector.tensor_tensor(acc[:], acc[:], sacc[:], ADD)
            nc.sync.dma_start(loss_out[:], acc[:])

    nc.compile()
    return nc


def _consts(Yw1, Yb1, Yw2, Yb2, Yw3, Yb3, Qw1, Qb1, Qw2, Qb2, Qw3, Qb3,
            n_steps, n_chunks):
    """Host-side constant packing. All fp32 numpy."""
    H, HH = 128, 64
    f = np.float32
    w1col = np.concatenate([Yw1[1, :], Qw1[1, :]]).astype(f)  # [128]
    w1row = np.concatenate([Yw1[0, :], Qw1[0, :]]).astype(f)
    b1 = np.concatenate([Yb1, Qb1]).astype(f)
    qb = f(DT) * Qb3.astype(f)[0]  # dt * Qb3

    l1c = np.zeros((n_chunks, n_chunks * H), f)
    for c in range(n_chunks):
        l1c[c, c * H : (c + 1) * H] = w1col

    w2d = np.zeros((H, H), f)
    w2d[:HH, :HH] = Yw2
    w2d[HH:, HH:] = Qw2

    w2p = (Yw1[1, :][:, None] * Yw2).astype(f)  # diag(Yw1col) @ Yw2

    yw3 = Yw3[:, 0].astype(f)
    qw3 = Qw3[:, 0].astype(f)
    C = n_chunks
    uc3 = np.zeros((HH, C * C), f)
    qc3 = np.zeros((H, C * C), f)
    for c in range(C):
        uc3[:, c * C + c] = yw3            # col c: u (and du) contraction
        qc3[HH:, c * C + c] = f(DT) * qw3  # col c: dt*q from h2_Q

    # t_n replicating reference's fp32 accumulation t += DT
    ts = np.zeros(n_steps + 1, f)
    t = f(0.0)
    for n in range(1, n_steps + 1):
        t = f(t + f(DT))
        ts[n] = t
    c1t = np.zeros((H, 64), f)
    for n in range(n_steps + 1):
        # qb drift fold: y_state = y_true - n*qb  =>  bias += n*qb*w1col
        c1t[:, n] = ts[n] * w1row + b1 + f(n) * qb * w1col

    s = f(np.sqrt(0.5 / DT))
    scal = np.zeros((128, 8), f)
    scal[:, 0] = s
    scal[:, 1] = s * qb
    scal[:, 2] = f(n_steps) * qb
    scal[:, 3] = Yb3.astype(f)[0]
    return dict(l1c=l1c, w2d=w2d, w2p=w2p, uc3=uc3, qc3=qc3, c1t=c1t,
                b2t=np.concatenate([Yb2, Qb2]).astype(f).reshape(H, 1),
                scal=scal)


def _run(dW, y0_init, weights, n_steps, n_cores, n_chunks, nfree,
         trace=False, tmpdir=None):
    f = np.float32
    B = dW.shape[1]
    bc = n_chunks * nfree
    assert B == n_cores * bc
    y0val = float(np.asarray(y0_init).reshape(-1)[0])

    key = (n_steps, n_chunks, nfree, y0val, ABL)
    if key not in _CACHE:
        _CACHE[key] = _build(n_steps, n_chunks, nfree, y0val)
    nc = _CACHE[key]

    cd = _consts(*weights, n_steps, n_chunks)
    dws = (f(SIGMA) * dW.reshape(n_steps, B)).astype(f)  # [S, B]

    in_maps = []
    for k in range(n_cores):
        m = dict(cd)
        m["dws"] = np.ascontiguousarray(
            dws[:, k * bc : (k + 1) * bc].reshape(n_steps, n_chunks, nfree))
        in_maps.append(m)

    global _LAST_RES
    res = run_bass_kernel_spmd(nc, in_maps, core_ids=list(range(n_cores)),
                               trace=trace, tmpdir=tmpdir)
    _LAST_RES = res
    total = f(0.0)
    for k in range(n_cores):
        total += res.results[k]["loss_part"].astype(np.float64).sum().astype(f)
    loss = np.float32(total / f(B))
    return np.asarray(loss, dtype=np.float32), res


def kernel(dW, y0_init, Yw1, Yb1, Yw2, Yb2, Yw3, Yb3,
           Qw1, Qb1, Qw2, Qb2, Qw3, Qb3):
    dW = np.asarray(dW, dtype=np.float32)
    weights = tuple(np.asarray(x, dtype=np.float32) for x in
                    (Yw1, Yb1, Yw2, Yb2, Yw3, Yb3, Qw1, Qb1, Qw2, Qb2, Qw3, Qb3))
    n_steps = dW.shape[0]
    B = dW.shape[1]
    # full-size path: 8 cores x 64 chunks x 512
    if B == B_TOTAL and n_steps == N_STEPS:
        out, _ = _run(dW, y0_init, weights, n_steps, N_CORES, N_CHUNKS, NFREE,
                      trace=bool(int(os.environ.get("FBSNN_TRACE", "0"))))
        return out
    # small/debug path: single core, scale chunks to B
    nfree = 512 if B % 512 == 0 else B
    n_chunks = B // nfree
    out, _ = _run(dW, y0_init, weights, n_steps, 1, n_chunks, nfree)
    return out



# revision 14
# speedup vs baseline: 2.6584x; 2.6584x over previous
"""FBSNN loss kernel for Trainium2 (8 NeuronCores, pure data parallel).

Reference: 50-step Euler scheme over B=262144 paths; each step evaluates a
Q-control MLP and a Y-value MLP (2->64->64->1, relu) plus dY/dy (JVP),
accumulating mean squared BSDE residuals and a terminal loss.

v2 kernel architecture (per core, Bc = 32768 paths = 64 chunks x 512):
  - Chunks processed in PAIRS (2p, 2p+1); every matmul is K=128 f32r with
    N=512 moving columns (~246 ns each; K=64 f32r runs ~2x slower).
  - y state kept DUPLICATED as y2 [128, 512] (rows 64:128 = rows 0:64) so
    the L1 one-hot extraction has K=128.
  - Per pair 8 matmuls: L1Y, L1Q (one-hot pair extraction x w1col outer),
    L2Y, L2Q (block-diag w2), v (relu-jvp through L2), u-gather, qd-gather
    (duplicated rows), du-gather. u and du accumulate into one psum bank
    (u rows 0:64, du rows 64:128) over all 32 pairs.
  - Elementwise split across engines: ACT does L1 relus, DVE does L2 relus
    (chained tensor_scalar add-bias/max) + masks, Pool (gpsimd TT) does
    dh2 = v * mask2 and the du*dW residual product.
  - Software-pipelined slot schedule: slot s runs L1(s), v/ug/qg(s-1),
    dug(s-2) on the PE so cross-engine deps never stall it.
  - Loss accumulated on-chip into [64, 1]; per-core partials summed on host.
"""

import os
import sys

import numpy as np

sys.path.insert(0, "/opt/trn_rl_repo")

import concourse.bacc as bacc  # noqa: E402
import concourse.tile as tile  # noqa: E402
from concourse import mybir  # noqa: E402
from concourse.bass_utils import run_bass_kernel_spmd  # noqa: E402

DT = 0.01
SIGMA = 0.5
N_STEPS = 50
N_CORES = 8
B_TOTAL = 262144
N_CHUNKS = 64
NFREE = 512
N_PAIRS = N_CHUNKS // 2
BC = N_CHUNKS * NFREE

F32 = mybir.dt.float32
F32R = mybir.dt.float32r
RELU = mybir.ActivationFunctionType.Relu
SIGN = mybir.ActivationFunctionType.Sign
SQUARE = mybir.ActivationFunctionType.Square
COPY = mybir.ActivationFunctionType.Copy
ADD = mybir.AluOpType.add
SUB = mybir.AluOpType.subtract
MULT = mybir.AluOpType.mult
MAXO = mybir.AluOpType.max
IS_GT = mybir.AluOpType.is_gt

_CACHE = {}
_LAST_RES = None
CFG = os.environ.get("FBSNN_CFG", "")


def _r(ap):
    return ap.bitcast(F32R)


def _build(n_steps, n_pairs, nfree, y0val):
    nc = bacc.Bacc(None, target_bir_lowering=False)
    P = n_pairs
    C = 2 * P  # chunks
    H = 128

    # ---- DRAM I/O ----
    dws = nc.dram_tensor("dws", [n_steps, C, nfree], F32, kind="ExternalInput")
    l1y = nc.dram_tensor("l1y", [H, P * H], F32R, kind="ExternalInput")
    l1q = nc.dram_tensor("l1q", [H, P * H], F32R, kind="ExternalInput")
    l2y = nc.dram_tensor("l2y", [H, H], F32R, kind="ExternalInput")
    l2q = nc.dram_tensor("l2q", [H, H], F32R, kind="ExternalInput")
    w2p2 = nc.dram_tensor("w2p2", [H, H], F32R, kind="ExternalInput")
    ug = nc.dram_tensor("ug", [H, P * C], F32R, kind="ExternalInput")
    qg2 = nc.dram_tensor("qg2", [H, P * H], F32R, kind="ExternalInput")
    c1ty = nc.dram_tensor("c1ty", [H, 64], F32, kind="ExternalInput")
    c1tq = nc.dram_tensor("c1tq", [H, 64], F32, kind="ExternalInput")
    b2y = nc.dram_tensor("b2y", [H, 1], F32, kind="ExternalInput")
    b2q = nc.dram_tensor("b2q", [H, 1], F32, kind="ExternalInput")
    scal = nc.dram_tensor("scal", [H, 8], F32, kind="ExternalInput")
    loss_out = nc.dram_tensor("loss_part", [C, 1], F32, kind="ExternalOutput")

    with tile.TileContext(nc) as tc:
        import contextlib

        with contextlib.ExitStack() as ctx:
            const = ctx.enter_context(tc.tile_pool(name="const", bufs=1))
            state = ctx.enter_context(tc.tile_pool(name="state", bufs=1))
            work = ctx.enter_context(tc.tile_pool(name="work", bufs=2))
            dwp = ctx.enter_context(tc.tile_pool(name="dwp", bufs=3))
            scr = ctx.enter_context(tc.tile_pool(name="scr", bufs=2))
            p_a1y = ctx.enter_context(tc.tile_pool(name="p_a1y", bufs=1, space="PSUM"))
            p_a1q = ctx.enter_context(tc.tile_pool(name="p_a1q", bufs=1, space="PSUM"))
            p_a2y = ctx.enter_context(tc.tile_pool(name="p_a2y", bufs=1, space="PSUM"))
            p_a2q = ctx.enter_context(tc.tile_pool(name="p_a2q", bufs=1, space="PSUM"))
            p_v = ctx.enter_context(tc.tile_pool(name="p_v", bufs=1, space="PSUM"))
            p_u = ctx.enter_context(tc.tile_pool(name="p_u", bufs=1, space="PSUM"))
            p_du = ctx.enter_context(tc.tile_pool(name="p_du", bufs=1, space="PSUM"))
            p_qd = ctx.enter_context(tc.tile_pool(name="p_qd", bufs=1, space="PSUM"))

            # ---- constants ----
            l1y_sb = const.tile([H, P * H], F32R)
            l1q_sb = const.tile([H, P * H], F32R)
            l2y_sb = const.tile([H, H], F32R)
            l2q_sb = const.tile([H, H], F32R)
            w2p2_sb = const.tile([H, H], F32R)
            ug_sb = const.tile([H, P * C], F32R)
            qg2_sb = const.tile([H, P * H], F32R)
            c1ty_sb = const.tile([H, 64], F32)
            c1tq_sb = const.tile([H, 64], F32)
            b2y_sb = const.tile([H, 1], F32)
            b2q_sb = const.tile([H, 1], F32)
            scal_sb = const.tile([H, 8], F32)
            for dst, src in ((l1y_sb, l1y), (l1q_sb, l1q), (l2y_sb, l2y),
                             (l2q_sb, l2q), (w2p2_sb, w2p2), (ug_sb, ug),
                             (qg2_sb, qg2), (c1ty_sb, c1ty), (c1tq_sb, c1tq),
                             (b2y_sb, b2y), (b2q_sb, b2q), (scal_sb, scal)):
                nc.sync.dma_start(dst[:], src[:])

            # ---- state ----
            y2 = state.tile([H, nfree], F32)
            udu_st = [state.tile([H, nfree], F32, tag=f"udu{p}", name=f"udu{p}")
                      for p in (0, 1)]
            acc = state.tile([C, 1], F32, tag="acc", name="acc")
            sacc = state.tile([C, 1], F32, tag="sacc", name="sacc")

            y_init = scr.tile([H, nfree], F32, tag="sq", name="y_init")
            nc.vector.memset(y_init[:], float(y0val))
            nc.scalar.activation(_r(y2[:]), y_init[:], COPY)
            nc.vector.memset(acc[:], 0.0)

            def emit_eval(e, par):
                """Eval the fused nets at step index e; write parity `par`."""
                by = c1ty_sb[:, e:e + 1]
                bq = c1tq_sb[:, e:e + 1]
                u_ps = p_u.tile([C, nfree], F32, tag="u_ps", name="u_ps")
                du_ps = p_du.tile([C, nfree], F32, tag="du_ps", name="du_ps")
                qd_ps = p_qd.tile([H, nfree], F32, tag="qd_ps", name="qd_ps")
                # rotating per-slot tiles, keyed by slot parity
                h1y_t = {}
                h1q_t = {}
                h2y_t = {}
                h2q_t = {}
                m1_t = {}
                dh2_t = {}
                v_ps = {}
                for s in range(P + 2):
                    # --- PE: L1 of pair s ---
                    if s < P:
                        a1y = p_a1y.tile([H, nfree], F32, tag="a1y", name="a1y")
                        a1q = p_a1q.tile([H, nfree], F32, tag="a1q", name="a1q")
                        nc.tensor.matmul(a1y[:], l1y_sb[:, s * H:(s + 1) * H],
                                         _r(y2[:]))
                        nc.tensor.matmul(a1q[:], l1q_sb[:, s * H:(s + 1) * H],
                                         _r(y2[:]))
                    q = s - 1
                    if 0 <= q < P:
                        # --- DVE: relu h2Y(q); ACT: relu h2Q(q) ---
                        h2y = work.tile([H, nfree], F32, tag="h2y", name="h2y")
                        h2q = work.tile([H, nfree], F32, tag="h2q", name="h2q")
                        h2y_t[q] = h2y
                        h2q_t[q] = h2q
                        nc.vector.tensor_scalar(
                            _r(h2y[:]), a2y_prev[:], b2y_sb[:, 0:1], 0.0,
                            ADD, MAXO)
                        if "h2qact" in CFG:
                            nc.scalar.activation(_r(h2q[:]), a2q_prev[:], RELU,
                                                 bias=b2q_sb[:, 0:1])
                        else:
                            nc.vector.tensor_scalar(
                                _r(h2q[:]), a2q_prev[:], b2q_sb[:, 0:1], 0.0,
                                ADD, MAXO)
                    r = s - 2
                    if 0 <= r < P:
                        # --- PE: du-gather of pair r ---
                        nc.tensor.matmul(
                            du_ps[:], ug_sb[:, r * C:(r + 1) * C],
                            _r(dh2_t.pop(r)[:]),
                            start=(r == 0), stop=(r == P - 1),
                            skip_group_check=True)
                    if 0 <= q < P:
                        # --- PE: v(q), gathers(q) ---
                        v = p_v.tile([H, nfree], F32, tag="v", name="v")
                        v_ps[q] = v
                        nc.tensor.matmul(v[:], w2p2_sb[:], _r(m1_t.pop(q)[:]))
                        nc.tensor.matmul(
                            u_ps[:], ug_sb[:, q * C:(q + 1) * C],
                            _r(h2y_t[q][:]),
                            start=(q == 0), stop=(q == P - 1),
                            skip_group_check=True)
                        nc.tensor.matmul(
                            qd_ps[:], qg2_sb[:, q * H:(q + 1) * H],
                            _r(h2q_t.pop(q)[:]),
                            start=(q == 0), stop=(q == P - 1),
                            skip_group_check=True)
                        # --- dh2(q) = v(q) * (h2y(q) > 0) ---
                        dh2 = work.tile([H, nfree], F32, tag="dh2", name="dh2")
                        dh2_t[q] = dh2
                        if "dhdve" in CFG:
                            # all-DVE: mask + psum-read multiply
                            m2 = work.tile([H, nfree], F32, tag="m2", name="m2")
                            nc.vector.tensor_scalar(m2[:], h2y_t.pop(q)[:],
                                                    0.0, None, IS_GT)
                            nc.vector.tensor_tensor(_r(dh2[:]), v_ps.pop(q)[:],
                                                    m2[:], MULT)
                        else:
                            # ACT evicts v to sbuf; DVE mask; Pool multiplies
                            m2 = work.tile([H, nfree], F32, tag="m2", name="m2")
                            v_sb = work.tile([H, nfree], F32, tag="v_sb",
                                             name="v_sb")
                            nc.scalar.activation(v_sb[:], v_ps.pop(q)[:], COPY)
                            nc.vector.tensor_scalar(m2[:], h2y_t.pop(q)[:],
                                                    0.0, None, IS_GT)
                            nc.gpsimd.tensor_tensor(_r(dh2[:]), v_sb[:],
                                                    m2[:], MULT)
                    if s < P:
                        # --- ACT: relu h1Y(s), h1Q(s) + mask1(s) Sign ---
                        h1y = work.tile([H, nfree], F32, tag="h1y", name="h1y")
                        h1q = work.tile([H, nfree], F32, tag="h1q", name="h1q")
                        m1 = work.tile([H, nfree], F32, tag="m1", name="m1")
                        h1y_t[s] = h1y
                        h1q_t[s] = h1q
                        m1_t[s] = m1
                        nc.scalar.activation(_r(h1y[:]), a1y[:], RELU, bias=by)
                        nc.scalar.activation(_r(h1q[:]), a1q[:], RELU, bias=bq)
                        if "m1act" in CFG:
                            nc.scalar.activation(_r(m1[:]), h1y[:], SIGN)
                        else:
                            nc.vector.tensor_scalar(_r(m1[:]), h1y[:], 0.0,
                                                    None, IS_GT)
                        # --- PE: L2 of pair s ---
                        a2y = p_a2y.tile([H, nfree], F32, tag="a2y", name="a2y")
                        a2q = p_a2q.tile([H, nfree], F32, tag="a2q", name="a2q")
                        nc.tensor.matmul(a2y[:], l2y_sb[:], _r(h1y_t.pop(s)[:]))
                        nc.tensor.matmul(a2q[:], l2q_sb[:], _r(h1q_t.pop(s)[:]))
                        a2y_prev = a2y
                        a2q_prev = a2q
                # evict u/du accumulation; qd stays in psum (consumed by the
                # y-update and sq before the next eval reuses the bank)
                nc.scalar.activation(udu_st[par][0:C, :], u_ps[:], COPY)
                nc.scalar.activation(udu_st[par][64:64 + C, :], du_ps[:], COPY)
                return qd_ps

            # ---- E_0 ----
            qd_prev = emit_eval(0, 0)

            # ---- main loop ----
            pool_tt = nc.vector.tensor_tensor if "resdve" in CFG \
                else nc.gpsimd.tensor_tensor
            s_scale = scal_sb[0:C, 0:1]
            for n in range(n_steps):
                src, dst = n % 2, (n + 1) % 2
                dw2 = dwp.tile([H, nfree], F32, tag="dw")
                nc.sync.dma_start(dw2[0:C, :], dws[n, :, :])
                nc.sync.dma_start(dw2[64:64 + C, :], dws[n, :, :])
                # sq uses this step's qd; compute before the psum bank is
                # recycled by the next eval
                sq = scr.tile([C, nfree], F32, tag="sq")
                nc.scalar.activation(sq[:], qd_prev[0:C, :], SQUARE,
                                     bias=scal_sb[0:C, 1:2], scale=s_scale)
                # y update (qb drift folded into c1t biases); qd read from psum
                nc.vector.tensor_tensor(_r(y2[:]), y2[:], qd_prev[:], ADD)
                pool_tt(_r(y2[:]), y2[:], dw2[:], ADD)
                qd_prev = emit_eval(n + 1, dst)
                # residual: r = (u1 - u0) + (0.5/dt)(qd+qb)^2 - du0*dW
                r1 = scr.tile([C, nfree], F32, tag="r1")
                r3 = scr.tile([C, nfree], F32, tag="r3")
                rr = scr.tile([C, nfree], F32, tag="rr")
                pool_tt(r1[:], udu_st[dst][0:C, :], udu_st[src][0:C, :], SUB)
                pool_tt(r3[:], udu_st[src][64:64 + C, :],
                        dw2[64:64 + C, :], MULT)
                pool_tt(r1[:], r1[:], sq[:], ADD)
                pool_tt(r1[:], r1[:], r3[:], SUB)
                nc.scalar.activation(rr[:], r1[:], SQUARE, accum_out=sacc[:])
                pool_tt(acc[:], acc[:], sacc[:], ADD)

            fin = n_steps % 2
            # ---- terminal: acc += (u_N + Yb3 - (y_N)^2)^2 ----
            t1 = scr.tile([C, nfree], F32, tag="sq")
            t2 = scr.tile([C, nfree], F32, tag="r1")
            rr = scr.tile([C, nfree], F32, tag="rr")
            nc.scalar.activation(t1[:], y2[0:C, :], SQUARE,
                                 bias=scal_sb[0:C, 2:3])
            nc.vector.tensor_tensor(t2[:], udu_st[fin][0:C, :], t1[:], SUB)
            nc.vector.tensor_scalar(t2[:], t2[:], scal_sb[0:C, 3:4], None, ADD)
            nc.scalar.activation(rr[:], t2[:], SQUARE, accum_out=sacc[:])
            nc.vector.tensor_tensor(acc[:], acc[:], sacc[:], ADD)
            nc.sync.dma_start(loss_out[:], acc[:])

    nc.compile()
    return nc


def _consts(Yw1, Yb1, Yw2, Yb2, Yw3, Yb3, Qw1, Qb1, Qw2, Qb2, Qw3, Qb3,
            n_steps, n_pairs):
    f = np.float32
    P = n_pairs
    C = 2 * P
    H = 128
    w1cY = Yw1[1, :].astype(f)
    w1rY = Yw1[0, :].astype(f)
    w1cQ = Qw1[1, :].astype(f)
    w1rQ = Qw1[0, :].astype(f)
    qb = f(DT) * Qb3.astype(f)[0]
    yw3 = Yw3[:, 0].astype(f)
    qw3 = Qw3[:, 0].astype(f)

    l1y = np.zeros((H, P * H), f)
    l1q = np.zeros((H, P * H), f)
    for p in range(P):
        l1y[2 * p, p * H:p * H + 64] = w1cY
        l1y[64 + 2 * p + 1, p * H + 64:p * H + 128] = w1cY
        l1q[2 * p, p * H:p * H + 64] = w1cQ
        l1q[64 + 2 * p + 1, p * H + 64:p * H + 128] = w1cQ

    l2y = np.zeros((H, H), f)
    l2y[:64, :64] = Yw2
    l2y[64:, 64:] = Yw2
    l2q = np.zeros((H, H), f)
    l2q[:64, :64] = Qw2
    l2q[64:, 64:] = Qw2
    w2p = (w1cY[:, None] * Yw2).astype(f)
    w2p2 = np.zeros((H, H), f)
    w2p2[:64, :64] = w2p
    w2p2[64:, 64:] = w2p

    ug = np.zeros((H, P * C), f)
    qg2 = np.zeros((H, P * H), f)
    for p in range(P):
        ug[0:64, p * C + 2 * p] = yw3
        ug[64:128, p * C + 2 * p + 1] = yw3
        qg2[0:64, p * H + 2 * p] = f(DT) * qw3
        qg2[64:128, p * H + 2 * p + 1] = f(DT) * qw3
        qg2[0:64, p * H + 64 + 2 * p] = f(DT) * qw3
        qg2[64:128, p * H + 64 + 2 * p + 1] = f(DT) * qw3

    ts = np.zeros(n_steps + 1, f)
    t = f(0.0)
    for n in range(1, n_steps + 1):
        t = f(t + f(DT))
        ts[n] = t
    c1ty = np.zeros((H, 64), f)
    c1tq = np.zeros((H, 64), f)
    for n in range(n_steps + 1):
        colY = ts[n] * w1rY + Yb1.astype(f) + f(n) * qb * w1cY
        colQ = ts[n] * w1rQ + Qb1.astype(f) + f(n) * qb * w1cQ
        c1ty[0:64, n] = colY
        c1ty[64:128, n] = colY
        c1tq[0:64, n] = colQ
        c1tq[64:128, n] = colQ

    s = f(np.sqrt(0.5 / DT))
    scal = np.zeros((H, 8), f)
    scal[:, 0] = s
    scal[:, 1] = s * qb
    scal[:, 2] = f(n_steps) * qb
    scal[:, 3] = Yb3.astype(f)[0]
    b2y = np.concatenate([Yb2, Yb2]).astype(f).reshape(H, 1)
    b2q = np.concatenate([Qb2, Qb2]).astype(f).reshape(H, 1)
    return dict(l1y=l1y, l1q=l1q, l2y=l2y, l2q=l2q, w2p2=w2p2, ug=ug,
                qg2=qg2, c1ty=c1ty, c1tq=c1tq, b2y=b2y, b2q=b2q, scal=scal)


def _run(dW, y0_init, weights, n_steps, n_cores, n_pairs, nfree,
         trace=False, tmpdir=None):
    f = np.float32
    B = dW.shape[1]
    C = 2 * n_pairs
    bc = C * nfree
    assert B == n_cores * bc
    y0val = float(np.asarray(y0_init).reshape(-1)[0])

    key = (n_steps, n_pairs, nfree, y0val, CFG)
    if key not in _CACHE:
        _CACHE[key] = _build(n_steps, n_pairs, nfree, y0val)
    nc = _CACHE[key]

    cd = _consts(*weights, n_steps, n_pairs)
    dws = (f(SIGMA) * dW.reshape(n_steps, B)).astype(f)

    in_maps = []
    for k in range(n_cores):
        m = dict(cd)
        m["dws"] = np.ascontiguousarray(
            dws[:, k * bc:(k + 1) * bc].reshape(n_steps, C, nfree))
        in_maps.append(m)

    global _LAST_RES
    res = run_bass_kernel_spmd(nc, in_maps, core_ids=list(range(n_cores)),
                               trace=trace, tmpdir=tmpdir)
    _LAST_RES = res
    total = f(0.0)
    for k in range(n_cores):
        total += res.results[k]["loss_part"].astype(np.float64).sum().astype(f)
    loss = np.float32(total / f(B))
    return np.asarray(loss, dtype=np.float32), res


def _kernel_numpy(dW, y0_init, Yw1, Yb1, Yw2, Yb2, Yw3, Yb3,
                  Qw1, Qb1, Qw2, Qb2, Qw3, Qb3):
    """Fallback for non-full shapes (not used by the harness)."""
    f = np.float32
    n_steps, B, _ = dW.shape
    y = np.broadcast_to(np.asarray(y0_init, f).reshape(1, -1), (B, 1)).copy()
    t = np.zeros((B, 1), f)

    def mlp(t, y, w1, b1, w2, b2, w3, b3):
        x = np.concatenate([t, y], axis=1)
        h = np.maximum(x @ w1 + b1, 0)
        h2 = np.maximum(h @ w2 + b2, 0)
        return h2 @ w3 + b3, h, h2

    def ynet(t, y):
        u, h1, h2 = mlp(t, y, Yw1, Yb1, Yw2, Yb2, Yw3, Yb3)
        m1 = (h1 > 0).astype(f)
        m2 = (h2 > 0).astype(f)
        du = ((m1 * Yw1[1]) @ Yw2 * m2) @ Yw3
        return u, du

    u0, du0 = ynet(t, y)
    loss = f(0.0)
    for n in range(n_steps):
        q, _, _ = mlp(t, y, Qw1, Qb1, Qw2, Qb2, Qw3, Qb3)
        y1 = y + q * f(DT) + f(SIGMA) * dW[n]
        z0 = f(SIGMA) * du0
        t1 = t + f(DT)
        u1, du1 = ynet(t1, y1)
        resid = u1 - (u0 - 0.5 * q * q * f(DT) + z0 * dW[n])
        loss = loss + (resid * resid).mean(dtype=np.float64).astype(f)
        t, y, u0, du0 = t1, y1, u1, du1
    term = u0 - y * y
    loss = loss + (term * term).mean(dtype=np.float64).astype(f)
    return np.float32(loss)


def kernel(dW, y0_init, Yw1, Yb1, Yw2, Yb2, Yw3, Yb3,
           Qw1, Qb1, Qw2, Qb2, Qw3, Qb3):
    dW = np.asarray(dW, dtype=np.float32)
    weights = tuple(np.asarray(x, dtype=np.float32) for x in
                    (Yw1, Yb1, Yw2, Yb2, Yw3, Yb3, Qw1, Qb1, Qw2, Qb2, Qw3, Qb3))
    n_steps = dW.shape[0]
    B = dW.shape[1]
    if B == B_TOTAL and n_steps == N_STEPS:
        out, _ = _run(dW, y0_init, weights, n_steps, N_CORES, N_PAIRS, NFREE,
                      trace=bool(int(os.environ.get("FBSNN_TRACE", "0"))))
        return out
    if B % (2 * NFREE) == 0 and n_steps <= 63:
        n_pairs = B // (2 * NFREE)
        out, _ = _run(dW, np.asarray(y0_init, np.float32), weights, n_steps,
                      1, n_pairs, NFREE)
        return out
    return _kernel_numpy(dW, np.asarray(y0_init, np.float32), *weights)
